# revision 10
# baseline (speedup 1.0000x reference)
# DeepGEMM-style fp8 block-quantized linear for Trainium2, 8-core SPMD.
#
# reference semantics:
#   x_dq = dequant(quant_e4m3fn(x, per-token per-128-group amax/448 scales))
#   w_dq = w_fp8 * w_scale (per 128x128 block)
#   out  = (x_dq @ w_dq.T).astype(bf16)          # fp32 accumulation
#
# Strategy (per core; 2x4 [M x N] grid => M2=2048, N2=1024 per core):
#   - TRN fp8_e4m3 tops out at 240 (vs OCP e4m3fn's 448), so quantize
#     x * (112/amax) on TRN's grid: identical rounding for normals (pure
#     exponent shift); dequantize with s4 = amax/112.
#   - scales folded into fp16 matmul operands (fp16 keeps the e4m3fn
#     values and 448-max weight values exact); x transposed to [K, M]
#     via the on-chip xbar DMA.
#   - W is shipped K-major (host-side layout transform, part of shard
#     staging), so it DMAs straight into its final [k%128, kb, n] SBUF
#     layout as 8 contiguous k-slabs (f32->f16 cast in the DGE) with no
#     on-chip transpose; block scales are applied in place per slab.
#   - phase A: 3 m-tiles' 12 psum quarter-tiles accumulate slab-by-slab
#     as W streams in, so the PE starts ~15us in and tracks the HBM
#     load; phase B streams the remaining m-tiles with the x-quant
#     pipeline emitted 2 m-tiles ahead of the matmuls.

import numpy as np
import ml_dtypes
from contextlib import ExitStack

import concourse.bass as bass
import concourse.mybir as mybir
import concourse.tile as tile
from concourse import bacc
from concourse.bass_utils import run_bass_kernel_spmd

dt = mybir.dt

M, N, K = 4096, 4096, 7168
MSH, NSH = 2, 4                     # core grid: 2 along M, 4 along N
NCORES = MSH * NSH
BLK = 128


def bcast_inner(ap, n):
    """Append a step-0 inner dim of size n (free-dim broadcast read)."""
    return bass.AP(tensor=ap.tensor, offset=ap.offset, ap=[*ap.ap, [0, n]])


def emit_kernel(ctx, tc, o_d, x_d, w_d, ws_d, *, nq_width=512, batch_a=3):
    nc = tc.nc
    f32, f16, f8 = dt.float32, dt.float16, dt.float8e4
    bf16 = dt.bfloat16
    M2, Kd = x_d.shape
    _, N2 = w_d.shape              # w_d is [K, N2] (K-major, host-transposed)
    KB = Kd // BLK                 # k-blocks
    NB = N2 // BLK                 # n-blocks
    MT = M2 // BLK                 # m-tiles
    NQ = N2 // nq_width            # psum tiles per m-tile
    KQ = 4                         # x pipeline chunks per m-tile
    KBQ = KB // KQ
    KL = Kd // KQ
    WB = [0, 2, 4, 7, 10, 14, 18, 23, 28, 35, 42, 49, 56]  # W slab bounds (kb)
    KC = len(WB) - 1
    assert KB % KQ == 0 and WB[-1] == KB

    wtp = ctx.enter_context(tc.tile_pool(name="wt", bufs=1))
    constp = ctx.enter_context(tc.tile_pool(name="consts", bufs=1))
    xnp = ctx.enter_context(tc.tile_pool(name="xn", bufs=6))
    scp = ctx.enter_context(tc.tile_pool(name="sc", bufs=8))
    xqp = ctx.enter_context(tc.tile_pool(name="xq", bufs=3))
    xdqp = ctx.enter_context(tc.tile_pool(name="xdq", bufs=3))
    xtp = ctx.enter_context(tc.tile_pool(name="xt", bufs=4 * batch_a))
    obp = ctx.enter_context(tc.tile_pool(name="ob", bufs=4))
    psp = ctx.enter_context(tc.tile_pool(name="ps", bufs=8, space="PSUM"))

    # w_scale broadcast across partitions via step-0 partition DMA read
    wsb = constp.tile([128, NB * KB], f32)
    ws_flat = ws_d.rearrange("a b -> (a b)")
    ws_b = bass.AP(tensor=ws_flat.tensor, offset=ws_flat.offset,
                   ap=[[0, 128], *ws_flat.ap])
    nc.scalar.dma_start(wsb[:], ws_b)

    wt_t = wtp.tile([128, KB, N2], f16)
    w_r = w_d.rearrange("(kb p) n -> p kb n", p=BLK)

    def wload(kc):
        nc.gpsimd.dma_start(
            wt_t[:, WB[kc]:WB[kc + 1], :],
            w_r[:, WB[kc]:WB[kc + 1], :])

    def wscale(kc, half):
        # in-place block-scale of slab kc, n-half `half` (0/1); engine split
        # keeps vector/gpsimd evenly loaded during phase A
        eng = nc.vector if half == 0 else nc.gpsimd
        nbh = NB // 2
        kbc = WB[kc + 1] - WB[kc]
        sl = wt_t[:, WB[kc]:WB[kc + 1], half * (N2 // 2):(half + 1) * (N2 // 2)]
        slg = sl.rearrange("p kb (nb c) -> p kb nb c", c=BLK)
        base = wsb[:, half * nbh * KB + WB[kc]: half * nbh * KB + WB[kc] + kbc]
        ws_ap = bass.AP(tensor=base.tensor, offset=base.offset,
                        ap=[base.ap[0], base.ap[1], [KB, nbh], [0, BLK]])
        eng.tensor_tensor(out=slg, in0=slg, in1=ws_ap, op=mybir.AluOpType.mult)

    def xchunk(mt, q):
        xn = xnp.tile([128, KL], bf16, tag="xn")
        nc.sync.dma_start(
            xn[:], x_d[mt * BLK:(mt + 1) * BLK, q * KL:(q + 1) * KL])
        xng = xn[:].rearrange("p (kb c) -> p kb c", c=BLK)

        amax = scp.tile([128, KBQ], f32, tag="amax")
        nc.vector.reduce_max(
            amax[:], xng, axis=mybir.AxisListType.X, apply_absolute_value=True)
        # s4 ~= max(amax, 1e-12)/112 (== 4x reference scale up to 1 ulp)
        s4 = scp.tile([128, KBQ], f32, tag="s4")
        nc.gpsimd.tensor_scalar(
            out=s4[:], in0=amax[:],
            scalar1=1e-12, scalar2=float(np.float32(1.0 / 112.0)),
            op0=mybir.AluOpType.max, op1=mybir.AluOpType.mult)
        inv4 = scp.tile([128, KBQ], f32, tag="inv4")
        nc.vector.reciprocal(inv4[:], s4[:])

        xq = xqp.tile([128, KL], f8, tag="xq")
        xqg = xq[:].rearrange("p (kb c) -> p kb c", c=BLK)
        nc.gpsimd.tensor_tensor(
            out=xqg, in0=xng, in1=bcast_inner(inv4[:], BLK),
            op=mybir.AluOpType.mult)
        xdq = xdqp.tile([128, KL], f16, tag="xdq")
        xdqg = xdq[:].rearrange("p (kb c) -> p kb c", c=BLK)
        nc.vector.tensor_tensor(
            out=xdqg, in0=xqg, in1=bcast_inner(s4[:], BLK),
            op=mybir.AluOpType.mult)

        xt_t = xtp.tile([128, KBQ, 128], f16, tag="xt")
        nc.sync.dma_start(xt_t[:], xdq[:], transpose=True)
        return xt_t

    def mm_group(xt_qs, ob, nq):
        pst = psp.tile([128, nq_width], f32, tag="ps")
        for kb in range(KB):
            nc.tensor.matmul(
                pst[:],
                xt_qs[kb // KBQ][:, kb % KBQ, :],
                wt_t[:, kb, nq * nq_width:(nq + 1) * nq_width],
                start=(kb == 0), stop=(kb == KB - 1))
        nc.scalar.copy(ob[:, nq * nq_width:(nq + 1) * nq_width], pst[:])

    def ob_store(mt, ob):
        nc.scalar.dma_start(o_d[mt * BLK:(mt + 1) * BLK, :], ob[:])

    # ---- emission ----
    # The Tile scheduler orders each engine's stream from a cost-model
    # simulation that mispredicts DMA contention, so anchor every
    # phase-A emission group with strictly monotone logical times: the
    # per-engine order then matches emission order.  W slab loads are
    # interleaved with the gpsimd-side scale TTs so at most ~3 slabs
    # are in flight: this keeps the DMA queue shallow (dispatches never
    # block the gpsimd engine) and leaves DMA engines free for the
    # latency-critical x loads and xbar transposes.
    seq = [0]

    def anc():
        seq[0] += 1
        return tc.tile_wait_until(0.4 * seq[0] / 1000)

    a_xts = [[None] * KQ for _ in range(batch_a)]

    with anc():
        wload(0)
        wload(1)
        wload(2)
    for q in range(KQ):
        with anc():
            for mt in range(batch_a):
                a_xts[mt][q] = xchunk(mt, q)
        with anc():
            wscale(q, 0)
            wscale(q, 1)
        with anc():
            wload(q + 3)
    for kc in range(KQ, KC):
        with anc():
            wscale(kc, 0)
            wscale(kc, 1)
        if kc + 3 < KC:
            with anc():
                wload(kc + 3)

    # phase A: 6 live psums accumulate slab-by-slab as W streams in
    a_ps = [[psp.tile([128, nq_width], f32, tag="ps", name=f"ps_a{mt}_{nq}")
             for nq in range(NQ)] for mt in range(batch_a)]
    for kc in range(KC):
        for mt in range(batch_a):
            for nq in range(NQ):
                for kb in range(WB[kc], WB[kc + 1]):
                    nc.tensor.matmul(
                        a_ps[mt][nq][:],
                        a_xts[mt][kb // KBQ][:, kb % KBQ, :],
                        wt_t[:, kb, nq * nq_width:(nq + 1) * nq_width],
                        start=(kb == 0), stop=(kb == KB - 1))
    a_obs = [obp.tile([128, N2], bf16, tag="ob", name=f"ob_a{i}")
             for i in range(batch_a)]
    for mt in range(batch_a):
        for nq in range(NQ):
            nc.scalar.copy(
                a_obs[mt][:, nq * nq_width:(nq + 1) * nq_width], a_ps[mt][nq][:])
        ob_store(mt, a_obs[mt])

    # phase B: chains scheduled ~1 m-tile period ahead of their matmuls
    for mt in range(batch_a, MT):
        with tc.tile_wait_until((60.0 + 24.3 * (mt - batch_a)) / 1000):
            xt_qs = [xchunk(mt, q) for q in range(KQ)]
        ob = obp.tile([128, N2], bf16, tag="ob")
        for nq in range(NQ):
            mm_group(xt_qs, ob, nq)
        ob_store(mt, ob)

def build_nc(m2, n2, k, **kw):
    nc = bacc.Bacc("TRN2", target_bir_lowering=False, debug=False, num_devices=NCORES)
    x_d = nc.dram_tensor("x", [m2, k], dt.bfloat16, kind="ExternalInput").ap()
    w_d = nc.dram_tensor("w", [k, n2], dt.float32, kind="ExternalInput").ap()
    ws_d = nc.dram_tensor("ws", [n2 // BLK, k // BLK], dt.float32, kind="ExternalInput").ap()
    o_d = nc.dram_tensor("o", [m2, n2], dt.bfloat16, kind="ExternalOutput").ap()
    with tile.TileContext(nc) as tc, ExitStack() as ctx:
        emit_kernel(ctx, tc, o_d, x_d, w_d, ws_d, **kw)
    nc.compile()
    return nc


_cache = {}


def _get_nc():
    if "nc" not in _cache:
        _cache["nc"] = build_nc(M // MSH, N // NSH, K)
    return _cache["nc"]


def kernel(input, weight_fp8, weight_scale, _trace=False, _trace_kwargs=None):
    input = np.asarray(input)
    if input.dtype != ml_dtypes.bfloat16:
        input = input.astype(ml_dtypes.bfloat16)
    weight_fp8 = np.asarray(weight_fp8, dtype=np.float32)
    weight_scale = np.asarray(weight_scale, dtype=np.float32)
    M2, N2 = M // MSH, N // NSH
    NSB = N2 // BLK

    # K-major layout for W (pure layout transform; values untouched)
    w_t = np.ascontiguousarray(weight_fp8.T)          # [K, N] f32

    in_maps = []
    for c in range(NCORES):
        mi, ni = divmod(c, NSH)
        in_maps.append({
            "x": np.ascontiguousarray(input[mi * M2:(mi + 1) * M2]),
            "w": np.ascontiguousarray(w_t[:, ni * N2:(ni + 1) * N2]),
            "ws": np.ascontiguousarray(weight_scale[ni * NSB:(ni + 1) * NSB]),
        })

    nc = _get_nc()
    kw = {}
    if _trace:
        kw = dict(trace=True, **(_trace_kwargs or {}))
    res = run_bass_kernel_spmd(nc, in_maps, core_ids=list(range(NCORES)), **kw)

    out = np.empty((M, N), dtype=ml_dtypes.bfloat16)
    for c in range(NCORES):
        mi, ni = divmod(c, NSH)
        out[mi * M2:(mi + 1) * M2, ni * N2:(ni + 1) * N2] = res.results[c]["o"]
    if _trace:
        return out, res
    return out
